# revision 1
# baseline (speedup 1.0000x reference)
"""Charge-equilibration kernel for Trainium2 (8 NeuronCores, SPMD).

Problem: 1024 molecules x 128 atoms. Per molecule build the augmented
Coulomb matrix and solve the (n+1)x(n+1) linear system; return partial
charges [131072] f32.

Strategy: data-parallel over molecules (128 per core). Per molecule the
SPD 128x128 block A is built on-chip and inverted with Newton-Schulz
iterations in product form (E <- E^2, X <- X + E X, one stacked
[E | X] 128x256 matmul per iteration), warm-started from the quadratic
polynomial X0 = alpha I + beta A with per-molecule beta from a Rayleigh
estimate of lambda_max. The charge-conservation row is handled via the
Schur complement:
    v = A^-1 e,  u = A^-1 1,
    lam = (Q + sum v) / (1 - sum u),  q = -(v + lam*u).
Iterative-refinement steps in fp32 polish v,u to reference accuracy.

Molecules are processed in software-pipelined groups of G so the PE /
DVE / ACT / Pool engine streams interleave across molecules.
"""

import os
import numpy as np

import concourse.bass as bass
import concourse.bacc as bacc
import concourse.tile as tile
import concourse.mybir as mybir
from concourse.bass_utils import run_bass_kernel_spmd
from concourse.masks import make_identity

dt = mybir.dt
AF = mybir.ActivationFunctionType
ALU = mybir.AluOpType

N_CORES = 8
B_MOL = 1024
N_ATOM = 128
MPC = B_MOL // N_CORES          # molecules per core = 128
DMA_CHUNK = 8                   # molecules per lhs/rhs DMA
LAM_CHUNK = 32                  # molecules per batched lambda/q pass
G = int(os.environ.get("KE_G", "4"))               # pipeline group size

QUAD_X0 = os.environ.get("KE_QUAD", "1") == "1"    # quadratic X0 (fewer iters)
K_NS = int(os.environ.get("KE_K_NS", "10" if QUAD_X0 else "14"))
NS_F32R = os.environ.get("KE_F32R", "1") == "1"    # fp32r NS matmuls
N_REFINE = int(os.environ.get("KE_REFINE", "3" if NS_F32R else "2"))
C_INV = 1.0 / 40.0              # X0 = c*I ; c <= 1/lambda_max (rowsum bound ~35)
ALPHA = 1.55 / 8.0              # quad X0 = alpha*I + beta*A ; bulk top B=8
RAY_SCALE = 1.02                # Rayleigh -> lambda_max overestimate factor
EPS_D2 = 1.0e-4                 # diagonal-safety epsilon added to all d^2
ACT_COPIES = int(os.environ.get("KE_ACT_COPIES", "10"))  # E-copies on ACT
PB_BUFS = int(os.environ.get("KE_PB", "2"))
PN_BUFS = int(os.environ.get("KE_PN", "4"))
PT_BUFS = int(os.environ.get("KE_PT", "2"))

_CACHE = {}


def _build_bass():
    nc = bacc.Bacc()
    f32 = dt.float32

    lhs = nc.declare_dram_parameter(
        "lhs_pack", [7, MPC * N_ATOM], f32, isOutput=False)
    rhs = nc.declare_dram_parameter(
        "rhs_pack", [7, MPC * 2 * N_ATOM], f32, isOutput=False)
    dvp = nc.declare_dram_parameter(
        "dv_pack", [N_ATOM, MPC], f32, isOutput=False)
    bpk = nc.declare_dram_parameter(
        "b_pack", [N_ATOM, 2 * MPC], f32, isOutput=False)
    qpk = nc.declare_dram_parameter("q_pack", [1, MPC], f32, isOutput=False)
    out = nc.declare_dram_parameter("out", [N_ATOM, MPC], f32, isOutput=True)

    sdt = dt.float32r if NS_F32R else dt.float32

    def as32(ap):
        return ap.bitcast(dt.float32) if NS_F32R else ap

    with tile.TileContext(nc) as tc:
        with (
            tc.tile_pool(name="const", bufs=1) as const,
            tc.tile_pool(name="core_in", bufs=1) as core_in,
            tc.tile_pool(name="lhs_in", bufs=3) as lhs_in,
            tc.tile_pool(name="rhs_in", bufs=3) as rhs_in,
            tc.tile_pool(name="bld", bufs=6) as bld,
            tc.tile_pool(name="amat", bufs=3 * G + 2) as amat,
            tc.tile_pool(name="spool", bufs=4 * G + 2) as spool,
            tc.tile_pool(name="wpool", bufs=2 * G) as wpool,
            tc.tile_pool(name="lamp", bufs=2) as lamp,
            tc.tile_pool(name="p_build", bufs=PB_BUFS, space="PSUM") as p_build,
            tc.tile_pool(name="p_ns", bufs=PN_BUFS, space="PSUM") as p_ns,
            tc.tile_pool(name="p_thin", bufs=PT_BUFS, space="PSUM") as p_thin,
        ):
            # ---- constants ----
            ident = const.tile([128, 128], f32)
            make_identity(nc, ident[:])
            cmask = const.tile([128, 128], f32)  # 1 - I
            nc.gpsimd.memset(cmask[:], 1.0)
            nc.gpsimd.affine_select(
                out=cmask[:], in_=cmask[:], compare_op=ALU.not_equal,
                fill=0.0, base=0, pattern=[[-1, 128]], channel_multiplier=1,
            )
            ones_col = const.tile([128, 1], f32)
            nc.gpsimd.memset(ones_col[:], 1.0)
            alpha_i = None
            if QUAD_X0:
                alpha_i = const.tile([128, 128], f32)   # alpha * I
                nc.gpsimd.tensor_scalar_mul(alpha_i[:], ident[:], ALPHA)

            # ---- whole-core small inputs ----
            dv_all = core_in.tile([N_ATOM, MPC], f32)
            nc.sync.dma_start(dv_all[:], dvp[:])
            b_all = core_in.tile([N_ATOM, 2 * MPC], f32)
            nc.sync.dma_start(b_all[:], bpk[:])
            q_all = core_in.tile([1, MPC], f32)
            nc.sync.dma_start(q_all[:], qpk[:])

            dma_tiles = {}

            def emit_dma(m):
                c = m // DMA_CHUNK
                lh_c = lhs_in.tile([7, DMA_CHUNK * N_ATOM], f32, tag="lh")
                nc.sync.dma_start(
                    lh_c[:],
                    lhs[:, c * DMA_CHUNK * N_ATOM:(c + 1) * DMA_CHUNK * N_ATOM])
                rh_c = rhs_in.tile([7, DMA_CHUNK * 2 * N_ATOM], f32, tag="rh")
                nc.sync.dma_start(
                    rh_c[:],
                    rhs[:, c * DMA_CHUNK * 2 * N_ATOM:
                        (c + 1) * DMA_CHUNK * 2 * N_ATOM])
                dma_tiles[c] = (lh_c, rh_c)

            def emit_build_a(m):
                """Stage A: distance/gamma matmul + DVE reciprocals."""
                if m % DMA_CHUNK == 0:
                    emit_dma(m)
                lh_c, rh_c = dma_tiles[m // DMA_CHUNK]
                mo = m % DMA_CHUNK

                p1 = p_build.tile([128, 256], f32, tag="p1")  # [d2 | 2*g2]
                nc.tensor.matmul(
                    p1[:],
                    lh_c[:, mo * N_ATOM:(mo + 1) * N_ATOM],
                    rh_c[:, mo * 2 * N_ATOM:(mo + 1) * 2 * N_ATOM],
                )
                ig = bld.tile([128, 128], f32, tag="ig")      # 1/(2 g^2)
                nc.vector.reciprocal_approx_fast(ig[:], p1[:, 128:256])
                tr = bld.tile([128, 256], f32, tag="tr")      # [arg^2 | 1/d^2]
                nc.vector.tensor_mul(tr[:, 0:128], p1[:, 0:128], ig[:])
                nc.vector.reciprocal_approx_fast(tr[:, 128:256], p1[:, 0:128])
                return {"m": m, "tr": tr}

            def emit_build_b(sts):
                """Stage B: batched ACT sqrt then erf (one table load each)."""
                for st in sts:
                    si = bld.tile([128, 256], f32, tag="si")  # [arg | 1/d]
                    nc.scalar.sqrt(si[:], st["tr"][:])
                    st["si"] = si
                for st in sts:
                    ef = bld.tile([128, 128], f32, tag="ef")
                    nc.scalar.activation(ef[:], st["si"][:, 0:128], AF.Erf)
                    st["ef"] = ef

            def emit_build_c1(sts):
                """Assemble A (Pool), T = I - alpha A, and the Rayleigh
                vectors v1 = A 1 (DVE row sums), v2 = A v1 (PE)."""
                vvg = bld.tile([128, 2 * G], f32, tag="vvg")
                for i, st in enumerate(sts):
                    m = st["m"]
                    si, ef = st.pop("si"), st.pop("ef")
                    ao = bld.tile([128, 128], f32, tag="ao")
                    nc.gpsimd.tensor_mul(ao[:], ef[:], si[:, 128:256])
                    a1 = bld.tile([128, 128], f32, tag="a1")
                    nc.gpsimd.tensor_mul(a1[:], ao[:], cmask[:])
                    dtile = bld.tile([128, 128], f32, tag="dtile")
                    nc.gpsimd.tensor_scalar_mul(
                        dtile[:], ident[:], dv_all[:, m:m + 1])
                    ax = amat.tile([128, 128], f32, tag="A")
                    nc.gpsimd.tensor_add(ax[:], dtile[:], a1[:])
                    st["ax"] = ax
                    if QUAD_X0:
                        naa = bld.tile([128, 128], f32, tag="naa")
                        nc.gpsimd.tensor_scalar_mul(naa[:], ax[:], -ALPHA)
                        tt = bld.tile([128, 128], f32, tag="tt")
                        nc.gpsimd.tensor_add(tt[:], naa[:], ident[:])
                        st["t"] = tt                      # I - alpha A
                        nc.vector.reduce_sum(
                            vvg[:, 2 * i:2 * i + 1], ax[:],
                            axis=mybir.AxisListType.X)    # v1
                        p3 = p_thin.tile([128, 2 * G], f32, tag="pt")
                        nc.tensor.matmul(
                            p3[:, 0:1], ax[:], vvg[:, 2 * i:2 * i + 1])
                        nc.vector.tensor_copy(
                            vvg[:, 2 * i + 1:2 * i + 2], p3[:, 0:1])  # v2
                        st["vvg"] = vvg

            def emit_build_c2(sts):
                """Batched Rayleigh scalar chain -> beta, -beta broadcast."""
                if not QUAD_X0:
                    return
                vvg = sts[0]["vvg"]
                p4 = p_thin.tile([1, 2 * G], f32, tag="pt")
                for i, st in enumerate(sts):
                    nc.tensor.matmul(
                        p4[0:1, 2 * i:2 * i + 2],
                        vvg[:, 2 * i:2 * i + 1], vvg[:, 2 * i:2 * i + 2])
                sc = bld.tile([1, 4 * G], f32, tag="sc")
                nc.vector.reciprocal_approx_fast(
                    sc[:, 0:G], p4[0:1, 0:2 * G:2])           # 1/n1
                nc.vector.scalar_tensor_tensor(
                    out=sc[:, G:2 * G], in0=p4[0:1, 1:2 * G:2],
                    scalar=RAY_SCALE, in1=sc[:, 0:G],
                    op0=ALU.mult, op1=ALU.mult)               # S
                nc.vector.reciprocal_approx_fast(
                    sc[:, 2 * G:3 * G], sc[:, G:2 * G])       # 1/S
                nc.vector.tensor_scalar_add(
                    sc[:, 3 * G:4 * G], sc[:, 2 * G:3 * G], -ALPHA)
                sc2 = bld.tile([1, 2 * G], f32, tag="sc2")
                nc.vector.tensor_mul(
                    sc2[:, 0:G], sc[:, 2 * G:3 * G], sc[:, 3 * G:4 * G])
                nc.vector.tensor_scalar_mul(
                    sc2[:, G:2 * G], sc2[:, 0:G], -1.0)       # -beta
                bbg = bld.tile([128, 2 * G], f32, tag="bbg")
                nc.gpsimd.partition_broadcast(bbg[:], sc2[:])
                for i, st in enumerate(sts):
                    st["bcol"] = bbg[:, i:i + 1]
                    st["nbcol"] = bbg[:, G + i:G + i + 1]

            def emit_build_c3(st):
                """A^2 matmul, E0 = T - beta A^2 (DVE), X0 (Pool)."""
                ax = st["ax"]
                s_sb = spool.tile([128, 256], sdt, tag="S")
                st["s"] = s_sb
                if not QUAD_X0:
                    nc.gpsimd.tensor_scalar_mul(
                        s_sb[:, 128:256], ident[:], C_INV)
                    nca = bld.tile([128, 128], f32, tag="nca")
                    nc.gpsimd.tensor_scalar_mul(nca[:], ax[:], -C_INV)
                    nc.gpsimd.tensor_add(s_sb[:, 0:128], nca[:], ident[:])
                    return
                p2 = p_build.tile([128, 256], f32, tag="p1")  # A^2
                nc.tensor.matmul(p2[:, 0:128], ax[:], ax[:])
                nc.vector.scalar_tensor_tensor(
                    out=s_sb[:, 0:128], in0=p2[:, 0:128], scalar=st["nbcol"],
                    in1=st.pop("t")[:], op0=ALU.mult, op1=ALU.add)  # E0
                ba = bld.tile([128, 128], f32, tag="ba")
                nc.gpsimd.tensor_scalar_mul(ba[:], ax[:], st["bcol"])
                nc.gpsimd.tensor_add(s_sb[:, 128:256], ba[:], alpha_i[:])

            def emit_ns_round(st, k):
                s_sb = st["s"]
                p5 = p_ns.tile([128, 256], f32, tag="p5")
                nc.tensor.matmul(p5[:], s_sb[:, 0:128], s_sb[:, 0:256])
                s_nx = spool.tile([128, 256], sdt, tag="S")
                if k < ACT_COPIES:
                    nc.scalar.copy(s_nx[:, 0:128], p5[:, 0:128])
                else:
                    nc.vector.tensor_copy(s_nx[:, 0:128], p5[:, 0:128])
                nc.vector.tensor_add(
                    s_nx[:, 128:256], p5[:, 128:256], as32(s_sb[:, 128:256]))
                st["s"] = s_nx

            # ---- batched finish stages over a whole group ----
            def fin_stage_w(fs):
                sts = fs["sts"]
                m0 = sts[0]["m"]
                p6 = p_thin.tile([128, 2 * G], f32, tag="pt")
                for i, st in enumerate(sts):
                    nc.tensor.matmul(
                        p6[:, 2 * i:2 * i + 2], as32(st["s"][:, 128:256]),
                        b_all[:, 2 * (m0 + i):2 * (m0 + i) + 2])
                w = wpool.tile([128, 2 * G], f32, tag="w")
                nc.vector.tensor_copy(w[:], p6[:])
                fs["w"] = w

            def fin_stage_refine(fs, j):
                sts = fs["sts"]
                m0 = sts[0]["m"]
                w = fs["w"]
                last = j == N_REFINE - 1
                p7 = p_thin.tile([128, 2 * G], f32, tag="pt")
                for i, st in enumerate(sts):
                    nc.tensor.matmul(
                        p7[:, 2 * i:2 * i + 2], st["ax"][:],
                        w[:, 2 * i:2 * i + 2])
                rsd = wpool.tile([128, 2 * G], f32, tag="rsd")
                nc.vector.tensor_sub(
                    rsd[:], b_all[:, 2 * m0:2 * m0 + 2 * G], p7[:])
                p8 = p_thin.tile([128, 2 * G], f32, tag="pt")
                for i, st in enumerate(sts):
                    nc.tensor.matmul(
                        p8[:, 2 * i:2 * i + 2], as32(st["s"][:, 128:256]),
                        rsd[:, 2 * i:2 * i + 2])
                if last:
                    wall, sums, mm0 = fs["wall"], fs["sums"], fs["mm0"]
                    nc.vector.tensor_add(
                        wall[:, 2 * mm0:2 * mm0 + 2 * G], w[:], p8[:])
                else:
                    w2 = wpool.tile([128, 2 * G], f32, tag="w")
                    nc.vector.tensor_add(w2[:], w[:], p8[:])
                    fs["w"] = w2

            def fin_stage_sums(fs):
                wall, sums, mm0 = fs["wall"], fs["sums"], fs["mm0"]
                if N_REFINE == 0:
                    nc.vector.tensor_copy(
                        wall[:, 2 * mm0:2 * mm0 + 2 * G], fs["w"][:])
                p9 = p_thin.tile([1, 2 * G], f32, tag="pt")
                nc.tensor.matmul(
                    p9[:], ones_col[:], wall[:, 2 * mm0:2 * mm0 + 2 * G])
                nc.vector.tensor_copy(
                    sums[:, 2 * mm0:2 * mm0 + 2 * G], p9[:])
                if fs["emit_lam"]:
                    emit_lambda(fs["lc"], wall, sums)

            def emit_lambda(lc, wall, sums):
                s0 = sums[:, 0:2 * LAM_CHUNK:2]
                s1 = sums[:, 1:2 * LAM_CHUNK:2]
                num = lamp.tile([1, LAM_CHUNK], f32, tag="num")
                nc.vector.tensor_add(
                    num[:], s0, q_all[:, lc * LAM_CHUNK:(lc + 1) * LAM_CHUNK])
                den = lamp.tile([1, LAM_CHUNK], f32, tag="den")
                nc.vector.tensor_scalar_add(den[:], s1, -1.0)
                rden = lamp.tile([1, LAM_CHUNK], f32, tag="rden")
                nc.vector.reciprocal(rden[:], den[:])
                lamneg = lamp.tile([1, LAM_CHUNK], f32, tag="lamneg")
                nc.vector.tensor_mul(lamneg[:], num[:], rden[:])
                lamb = lamp.tile([128, LAM_CHUNK], f32, tag="lamb")
                nc.gpsimd.partition_broadcast(lamb[:], lamneg[:])
                vall = wall[:, 0:2 * LAM_CHUNK:2]
                uall = wall[:, 1:2 * LAM_CHUNK:2]
                t1 = lamp.tile([128, LAM_CHUNK], f32, tag="t1")
                nc.vector.tensor_mul(t1[:], uall, lamb[:])   # -lam*u
                qc = lamp.tile([128, LAM_CHUNK], f32, tag="qc")
                nc.vector.tensor_sub(qc[:], t1[:], vall)     # -(v + lam u)
                nc.sync.dma_start(
                    out[:, lc * LAM_CHUNK:(lc + 1) * LAM_CHUNK], qc[:])

            # ---- software-pipelined emission over molecule groups ----
            n_groups = MPC // G
            wall_t = {}

            def build_stage_iter(ms):
                sts = [emit_build_a(m) for m in ms]
                yield
                emit_build_b(sts)
                yield
                emit_build_c1(sts)
                yield
                emit_build_c2(sts)
                yield
                for s in sts:
                    emit_build_c3(s)
                yield sts

            def fin_stage_iter(sts):
                m0 = sts[0]["m"]
                lc = m0 // LAM_CHUNK
                if m0 % LAM_CHUNK == 0:
                    wall_new = lamp.tile(
                        [128, 2 * LAM_CHUNK], f32, tag="wall")
                    sums_new = lamp.tile([1, 2 * LAM_CHUNK], f32, tag="sums")
                    wall_t[lc] = (wall_new, sums_new)
                wall, sums = wall_t[lc]
                fs = {"sts": sts, "wall": wall, "sums": sums,
                      "mm0": m0 % LAM_CHUNK, "lc": lc,
                      "emit_lam": (m0 % LAM_CHUNK) + G == LAM_CHUNK}
                fin_stage_w(fs)
                yield
                for j in range(N_REFINE):
                    fin_stage_refine(fs, j)
                    yield
                fin_stage_sums(fs)
                yield

            def drain(it):
                if it is None:
                    return None
                ret = None
                for v in it:
                    if v is not None:
                        ret = v
                return ret

            bi = build_stage_iter(range(G))
            st_map = {}
            sts_cur = drain(bi)
            fi = None
            for g in range(n_groups):
                bi = build_stage_iter(
                    range((g + 1) * G, (g + 2) * G))                     if g + 1 < n_groups else None
                built = None
                for k in range(K_NS):
                    for s in sts_cur:
                        emit_ns_round(s, k)
                    if k % 2 == 0 and bi is not None:
                        built = next(bi, None) or built
                    elif k % 2 == 1 and fi is not None:
                        next(fi, None)
                if bi is not None:
                    built = drain(bi) or built
                if fi is not None:
                    drain(fi)
                fi = fin_stage_iter(sts_cur)
                sts_cur = built
            drain(fi)

    nc.compile()
    return nc


def _host_pack(eneg, positions, node_attrs, hardness, total_charge,
               atomic_numbers):
    """Precompute per-atom quantities and pack per-core DRAM tensors."""
    f32 = np.float32
    pos = np.ascontiguousarray(positions, dtype=f32).reshape(B_MOL, N_ATOM, 3)
    Z = np.asarray(atomic_numbers).astype(np.int64).reshape(B_MOL, N_ATOM)
    na = np.asarray(node_attrs, dtype=f32).reshape(B_MOL, N_ATOM, -1)
    hard = np.asarray(hardness, dtype=f32)
    e = np.asarray(eneg, dtype=f32).reshape(B_MOL, N_ATOM)
    Q = np.asarray(total_charge, dtype=f32).reshape(B_MOL)

    cov = (0.3 + 0.02 * np.arange(100)).astype(f32)
    r = cov[Z]                                   # [B, n]
    sig2 = (f32(2.0) * r * r).astype(f32)        # 2*sigma
    n2 = (pos * pos).sum(axis=2, dtype=f32).astype(f32)
    aidx = na.argmax(axis=2)
    dv = (hard[aidx] + f32(1.0) / (np.sqrt(np.pi).astype(f32) * r)).astype(f32)

    mpc = MPC
    in_maps = []
    for c in range(N_CORES):
        sl = slice(c * mpc, (c + 1) * mpc)
        p = pos[sl]          # [mpc, 128, 3]
        nn2 = n2[sl]         # [mpc, 128]
        ss = sig2[sl]        # [mpc, 128]
        lhsp = np.zeros((7, mpc, N_ATOM), dtype=f32)
        lhsp[0] = -2.0 * p[:, :, 0]
        lhsp[1] = -2.0 * p[:, :, 1]
        lhsp[2] = -2.0 * p[:, :, 2]
        lhsp[3] = nn2
        lhsp[4] = 1.0
        lhsp[5] = ss
        lhsp[6] = EPS_D2
        rhsp = np.zeros((7, mpc, 2 * N_ATOM), dtype=f32)
        rhsp[0, :, :N_ATOM] = p[:, :, 0]
        rhsp[1, :, :N_ATOM] = p[:, :, 1]
        rhsp[2, :, :N_ATOM] = p[:, :, 2]
        rhsp[3, :, :N_ATOM] = 1.0
        rhsp[4, :, :N_ATOM] = nn2
        rhsp[6, :, :N_ATOM] = 1.0
        rhsp[4, :, N_ATOM:] = ss
        rhsp[5, :, N_ATOM:] = 1.0
        dvp = np.ascontiguousarray(dv[sl].T)
        bp = np.empty((N_ATOM, 2 * mpc), dtype=f32)
        bp[:, 0::2] = e[sl].T
        bp[:, 1::2] = 1.0
        qp = np.ascontiguousarray(Q[sl]).reshape(1, mpc)
        in_maps.append({
            "lhs_pack": np.ascontiguousarray(lhsp.reshape(7, mpc * N_ATOM)),
            "rhs_pack": np.ascontiguousarray(rhsp.reshape(7, mpc * 2 * N_ATOM)),
            "dv_pack": dvp,
            "b_pack": bp,
            "q_pack": qp,
        })
    return in_maps


def run_device(in_maps, trace=False, **kw):
    if "nc" not in _CACHE:
        _CACHE["nc"] = _build_bass()
    nc = _CACHE["nc"]
    return run_bass_kernel_spmd(nc, in_maps, list(range(N_CORES)),
                                trace=trace, **kw)


def kernel(eneg, positions, node_attrs, hardness, total_charge, batch,
           atomic_numbers):
    in_maps = _host_pack(eneg, positions, node_attrs, hardness, total_charge,
                         atomic_numbers)
    res = run_device(in_maps)
    outs = []
    for c in range(N_CORES):
        o = res.results[c]["out"]                # [atom, mol]
        outs.append(np.ascontiguousarray(o.T))   # [mol, atom]
    full = np.concatenate(outs, axis=0).reshape(-1).astype(np.float32)
    return full



# revision 31
# speedup vs baseline: 2.4237x; 2.4237x over previous
"""Charge-equilibration kernel for Trainium2 (8 NeuronCores, SPMD).

Problem: 1024 molecules x 128 atoms. Per molecule build the Coulomb matrix
A (erf-screened), solve the augmented system via Schur complement, return
partial charges [131072] f32.

Algorithm (per core: 128 molecules, data-parallel across cores):
  - Symmetric Jacobi scaling At = ss^T o A (ss = 1/sqrt(diag A), host-packed
    with a global spectral scale 1/S0 folded in) so diag(At/S0) = 1/S0 and
    all Newton-Schulz warm-start coefficients are global constants.
  - Quadratic warm start X0 = k4*I + b'*G, E0 = k1*I + k2*G + k3*G^2 with
    G = bf16(ss^T o A_off / S0); then K-1 product-form NS rounds
    (E<-E^2, X<-X+EX, both via PE with PSUM accumulation: X' = I@X + E@X)
    in bf16, plus a final X-only round.
  - w = X_K @ bt, then R iterative-refinement steps against the exact f32
    off-diagonal matrix ao with thin per-cohort ops.
  - Schur: lam = (Q + sum v)/(1 - sum u), q = -(v + lam*u).

Emission is software-pipelined over cohorts of 8 quads (32 molecules) with
phase-batched ACT table usage (sqrt phase, erf phase) so activation-table
reloads are amortized.
"""

import os
import numpy as np

import concourse.bass as bass
import concourse.bacc as bacc
import concourse.tile as tile
import concourse.mybir as mybir
from concourse.bass_utils import run_bass_kernel_spmd
from concourse.masks import make_identity

dt = mybir.dt
AF = mybir.ActivationFunctionType
ALU = mybir.AluOpType

N_CORES = 8
B_MOL = 1024
N_ATOM = 128
MPC = B_MOL // N_CORES          # molecules per core = 128
QG = 4                          # molecules per quad
NQ = MPC // QG                  # 32 quads
CQ = 8                          # quads per cohort (= LAM chunk of 32 mols)
NCOH = NQ // CQ                 # 4 cohorts
DMA_CHUNK = 4                   # molecules per lhs/rhs/sst DMA

S0 = float(os.environ.get("KE_S0", "35.0"))
APRIME = float(os.environ.get("KE_AP", "6.79"))
BPRIME = 1.0 - APRIME
C0 = 1.0 / S0
K1C = float(1.0 - APRIME * C0 - BPRIME * C0 * C0)
K2C = float(-(APRIME + 2.0 * BPRIME * C0))
K3C = float(-BPRIME)
K4C = float(APRIME + BPRIME * C0)
K_NS = int(os.environ.get("KE_K", "7"))     # total rounds (last is X-only)
N_REF = int(os.environ.get("KE_R", "3"))
OFF = int(os.environ.get("KE_OFF", "11"))   # cohort pipeline offset (ticks)
EPS_D2 = 1.0e-4
L_DIAG = float(os.environ.get("KE_L", "1e10"))
SQRT_L = float(np.sqrt(L_DIAG))

# engine letters: a=ACT, v=DVE, p=Pool(gpsimd)
# note: Pool (gpsimd) cannot read PSUM - PSUM consumers must be a/v.
ECOPY = os.environ.get("KE_ECOPY", "aavaav")     # per squaring round (K_NS-1)
E_AO = os.environ.get("KE_AO", "v")
E_G = os.environ.get("KE_G", "p")
E_T = os.environ.get("KE_T", "v")       # stt: DVE only
E_ARG2 = os.environ.get("KE_ARG2", "v")
E_U1 = os.environ.get("KE_U1", "p")
E_T2 = os.environ.get("KE_T2", "v")     # stt: DVE only
E_RT = os.environ.get("KE_RT", "p")

_CACHE = {}


def _build_bass():
    nc = bacc.Bacc()
    f32 = dt.float32
    bf = dt.bfloat16

    lhs = nc.declare_dram_parameter("lhs_pack", [7, MPC * N_ATOM], f32,
                                    isOutput=False)
    rhs = nc.declare_dram_parameter("rhs_pack", [7, MPC * 2 * N_ATOM], f32,
                                    isOutput=False)
    sst = nc.declare_dram_parameter("sst_pack", [N_ATOM, MPC * N_ATOM], bf,
                                    isOutput=False)
    btp = nc.declare_dram_parameter("bt_pack", [N_ATOM, 2 * MPC], f32,
                                    isOutput=False)
    dv2 = nc.declare_dram_parameter("dvs2_pack", [N_ATOM, 2 * MPC], f32,
                                    isOutput=False)
    dvS = nc.declare_dram_parameter("dvsS2_pack", [N_ATOM, 2 * MPC], f32,
                                    isOutput=False)
    qpk = nc.declare_dram_parameter("q_pack", [1, MPC], f32, isOutput=False)
    out = nc.declare_dram_parameter("out", [N_ATOM, MPC], f32, isOutput=True)
    DBG = os.environ.get("KE_DBG", "") == "1"
    dbg = {}
    if DBG:
        for nm in ["d_sqin", "d_ao", "d_g", "d_e0", "d_sx", "d_e1", "d_w0"]:
            w = 1024 if nm == "d_sqin" else 512
            if nm == "d_w0":
                w = 64
            dbg[nm] = nc.declare_dram_parameter(nm, [N_ATOM, w], f32,
                                                isOutput=True)

    def cp(eng, dst, src):
        if eng == "a":
            nc.scalar.copy(dst, src)
        elif eng == "v":
            nc.vector.tensor_copy(dst, src)
        else:
            nc.gpsimd.tensor_copy(dst, src)

    def mul(eng, dst, a, b):
        (nc.vector if eng == "v" else nc.gpsimd).tensor_mul(dst, a, b)

    def stt(eng, dst, in0, scalar, in1, op0=ALU.mult, op1=ALU.add):
        (nc.vector if eng == "v" else nc.gpsimd).scalar_tensor_tensor(
            out=dst, in0=in0, scalar=scalar, in1=in1, op0=op0, op1=op1)

    from contextlib import ExitStack

    with tile.TileContext(nc) as tc:
        with ExitStack() as es:
            def pool(name, bufs, space=None):
                kw = {"space": space} if space else {}
                return es.enter_context(
                    tc.tile_pool(name=name, bufs=bufs, **kw))

            const = pool("const", 1)
            core_in = pool("core_in", 1)
            lhs_in = pool("lhs_in", 3)
            rhs_in = pool("rhs_in", 3)
            sst_in = pool("sst_in", 10)
            sqp = pool("sqp", 6)
            w2p = pool("w2p", 2)
            aop = pool("aop", 10)
            gp = pool("gp", 2)
            sep = pool("sep", (K_NS + 1) * CQ + 4)
            sxp = pool("sxp", CQ + 3)
            cvp = pool("cvp", 4)
            wp = pool("wp", 5)
            fint = pool("fint", 6)
            lamp = pool("lamp", 8)
            p_build = pool("p_build", 2, "PSUM")
            p_warm = pool("p_warm", 1, "PSUM")
            p_e = pool("p_e", 2, "PSUM")
            p_thin = pool("p_thin", 3, "PSUM")
            # ---- constants ----
            ident_b = const.tile([128, 128], bf)
            make_identity(nc, ident_b[:])
            identL = const.tile([128, 128], bf)
            nc.gpsimd.memset(identL[:], SQRT_L)
            nc.gpsimd.affine_select(
                out=identL[:], in_=identL[:], compare_op=ALU.is_equal,
                fill=0.0, base=0, pattern=[[-1, 128]], channel_multiplier=1)
            k1i = const.tile([128, 4 * 128], bf)
            nc.gpsimd.memset(k1i[:], K1C)
            nc.gpsimd.affine_select(
                out=k1i[:], in_=k1i[:], compare_op=ALU.is_equal,
                fill=0.0, base=0, pattern=[[0, 4], [-1, 128]],
                channel_multiplier=1)
            k4i = const.tile([128, 4 * 128], bf)
            nc.gpsimd.memset(k4i[:], K4C)
            nc.gpsimd.affine_select(
                out=k4i[:], in_=k4i[:], compare_op=ALU.is_equal,
                fill=0.0, base=0, pattern=[[0, 4], [-1, 128]],
                channel_multiplier=1)
            ones_col = const.tile([128, 1], f32)
            nc.gpsimd.memset(ones_col[:], 1.0)

            # ---- whole-core small inputs ----
            bt_all = core_in.tile([N_ATOM, 2 * MPC], f32)
            nc.sync.dma_start(bt_all[:], btp[:])
            dv2_all = core_in.tile([N_ATOM, 2 * MPC], f32)
            nc.sync.dma_start(dv2_all[:], dv2[:])
            dvS_all = core_in.tile([N_ATOM, 2 * MPC], f32)
            nc.sync.dma_start(dvS_all[:], dvS[:])
            q_all = core_in.tile([1, MPC], f32)
            nc.sync.dma_start(q_all[:], qpk[:])
            btb_all = core_in.tile([N_ATOM, 2 * MPC], bf)
            nc.vector.tensor_copy(btb_all[:], bt_all[:])

            dma_tiles = {}

            def emit_dma(ch):
                lh_c = lhs_in.tile([7, DMA_CHUNK * N_ATOM], f32, tag="lh")
                nc.sync.dma_start(
                    lh_c[:], lhs[:, ch * DMA_CHUNK * N_ATOM:
                                  (ch + 1) * DMA_CHUNK * N_ATOM])
                rh_c = rhs_in.tile([7, DMA_CHUNK * 2 * N_ATOM], f32, tag="rh")
                nc.sync.dma_start(
                    rh_c[:], rhs[:, ch * DMA_CHUNK * 2 * N_ATOM:
                                  (ch + 1) * DMA_CHUNK * 2 * N_ATOM])
                ss_c = sst_in.tile([128, DMA_CHUNK * N_ATOM], bf, tag="ss")
                nc.sync.dma_start(
                    ss_c[:], sst[:, ch * DMA_CHUNK * N_ATOM:
                                  (ch + 1) * DMA_CHUNK * N_ATOM])
                dma_tiles[ch] = (lh_c, rh_c, ss_c)

            # ---- phases ----
            # cohort state: st[c] dict
            def ph_pre(st, c):
                st["sqin"] = {}
                st["ao"] = {}
                for qi in range(CQ):
                    q = c * CQ + qi
                    if (q * QG) % DMA_CHUNK == 0:
                        emit_dma((q * QG) // DMA_CHUNK)
                    ch = (q * QG) // DMA_CHUNK
                    lh_c, rh_c, ss_c = dma_tiles[ch]
                    bankD = p_build.tile([128, 512], f32, tag="b")
                    bankG = p_build.tile([128, 512], f32, tag="b")
                    for i in range(QG):
                        m = q * QG + i
                        mo = m % DMA_CHUNK
                        lsl = lh_c[:, mo * 128:(mo + 1) * 128]
                        first = i == 0
                        nc.tensor.matmul(
                            bankD[:, i * 128:(i + 1) * 128], lsl,
                            rh_c[:, mo * 256:mo * 256 + 128],
                            start=first, stop=False)
                        nc.tensor.matmul(
                            bankD[:, i * 128:(i + 1) * 128], identL[:],
                            identL[:], start=False, stop=(i == QG - 1))
                        nc.tensor.matmul(
                            bankG[:, i * 128:(i + 1) * 128], lsl,
                            rh_c[:, mo * 256 + 128:(mo + 1) * 256],
                            start=first, stop=(i == QG - 1))
                    sqin = sqp.tile([128, 1024], f32, tag="sq")
                    w2 = w2p.tile([128, 512], f32, tag="w2")
                    nc.vector.reciprocal_approx_fast(w2[:], bankG[:])
                    nc.vector.reciprocal_approx_fast(
                        sqin[:, 512:1024], bankD[:])
                    mul(E_ARG2, sqin[:, 0:512], bankD[:], w2[:])
                    st["sqin"][qi] = sqin

            def ph_sqrt(st, c):
                for qi in range(CQ):
                    sqin = st["sqin"][qi]
                    nc.scalar.sqrt(sqin[:], sqin[:])    # in-place

            def ph_erf(st, c):
                for qi in range(CQ):
                    sqin = st["sqin"][qi]
                    nc.scalar.activation(
                        sqin[:, 0:512], sqin[:, 0:512], AF.Erf)  # in-place
                for qi in range(CQ):
                    sqin = st["sqin"].pop(qi)
                    ao = aop.tile([128, 512], f32, tag="ao")
                    mul(E_AO, ao[:], sqin[:, 0:512], sqin[:, 512:1024])
                    st["ao"][qi] = ao
                    if DBG and c == 0 and qi == 0:
                        nc.sync.dma_start(dbg["d_sqin"][:], sqin[:])
                        nc.sync.dma_start(dbg["d_ao"][:], ao[:])

            def ph_warm(st, c):
                st["es"] = {}
                st["sx"] = {}
                for qi in range(CQ):
                    q = c * CQ + qi
                    ch = (q * QG) // DMA_CHUNK
                    ss_c = dma_tiles[ch][2]
                    qo = ((q * QG) % DMA_CHUNK) * 128
                    g = gp.tile([128, 512], bf, tag="g")
                    mul(E_G, g[:], st["ao"][qi][:], ss_c[:, qo:qo + 512])
                    t = gp.tile([128, 512], bf, tag="t")
                    stt(E_T, t[:], g[:], K2C, k1i[:])
                    pb = p_warm.tile([128, 512], f32, tag="bP")
                    for i in range(QG):
                        nc.tensor.matmul(
                            pb[:, i * 128:(i + 1) * 128],
                            g[:, i * 128:(i + 1) * 128],
                            g[:, i * 128:(i + 1) * 128],
                            start=(i == 0), stop=(i == QG - 1))
                    se = sep.tile([128, 512], bf, tag="se")
                    nc.vector.scalar_tensor_tensor(
                        out=se[:], in0=pb[:], scalar=K3C, in1=t[:],
                        op0=ALU.mult, op1=ALU.add)
                    sx = sxp.tile([128, 512], bf, tag="sx")
                    stt("v", sx[:], g[:], BPRIME, k4i[:])
                    st["es"][qi] = [se]
                    st["sx"][qi] = sx
                    if DBG and c == 0 and qi == 0:
                        tmpg = gp.tile([128, 512], f32, tag="dbg")
                        nc.vector.tensor_copy(tmpg[:], g[:])
                        nc.sync.dma_start(dbg["d_g"][:], tmpg[:])
                        tmpe = gp.tile([128, 512], f32, tag="dbg")
                        nc.vector.tensor_copy(tmpe[:], se[:])
                        nc.sync.dma_start(dbg["d_e0"][:], tmpe[:])
                        tmpx = gp.tile([128, 512], f32, tag="dbg")
                        nc.vector.tensor_copy(tmpx[:], sx[:])
                        nc.sync.dma_start(dbg["d_sx"][:], tmpx[:])

            def ph_ns(st, c, k):
                # squaring round: E_{k+1} = E_k^2 (per quad), store the chain
                for qi in range(CQ):
                    se = st["es"][qi][-1]
                    eb = p_e.tile([128, 512], f32, tag="eb")
                    for i in range(QG):
                        sl = slice(i * 128, (i + 1) * 128)
                        nc.tensor.matmul(eb[:, sl], se[:, sl], se[:, sl],
                                         start=(i == 0), stop=(i == QG - 1))
                    se2 = sep.tile([128, 512], bf, tag="se")
                    cp(ECOPY[k], se2[:], eb[:])
                    st["es"][qi].append(se2)
                    if DBG and c == 0 and qi == 0 and k == 0:
                        tmpe = gp.tile([128, 512], f32, tag="dbg")
                        nc.vector.tensor_copy(tmpe[:], se2[:])
                        nc.sync.dma_start(dbg["d_e1"][:], tmpe[:])

            def emit_apply(st, c, rhs_ap, rhs_bf, w_prev):
                """w_out = (w_prev +) chain_apply(rhs): X0 rhs then K-1
                (I+E_j) stages; thin cohort-wide ops."""
                nm = CQ * QG
                gb = p_thin.tile([128, 2 * nm], f32, tag="t")
                for mi in range(nm):
                    sx = st["sx"][mi // QG]
                    sl = slice((mi % QG) * 128, (mi % QG + 1) * 128)
                    nc.tensor.matmul(
                        gb[:, 2 * mi:2 * mi + 2], sx[:, sl],
                        rhs_bf[:, 2 * mi:2 * mi + 2],
                        start=(mi == 0), stop=(mi == nm - 1))
                cv = cvp.tile([128, 2 * nm], bf, tag="cv")
                nc.vector.tensor_copy(cv[:], gb[:])
                nk = len(st["es"][0])
                for j in range(nk):
                    cb = p_thin.tile([128, 2 * nm], f32, tag="t")
                    for mi in range(nm):
                        ej = st["es"][mi // QG][j]
                        sl = slice((mi % QG) * 128, (mi % QG + 1) * 128)
                        nc.tensor.matmul(
                            cb[:, 2 * mi:2 * mi + 2], ej[:, sl],
                            cv[:, 2 * mi:2 * mi + 2],
                            start=(mi == 0), stop=(mi == nm - 1))
                    if j < nk - 1:
                        cv2 = cvp.tile([128, 2 * nm], bf, tag="cv")
                        nc.vector.tensor_add(cv2[:], cv[:], cb[:])
                        cv = cv2
                    else:
                        w = wp.tile([128, 2 * nm], f32, tag="w")
                        if w_prev is None:
                            nc.vector.tensor_add(w[:], cv[:], cb[:])
                        else:
                            cv3 = fint.tile([128, 2 * nm], f32, tag="cv3")
                            nc.vector.tensor_add(cv3[:], cv[:], cb[:])
                            nc.vector.tensor_add(w[:], w_prev[:], cv3[:])
                return w

            def ph_fa(st, c):
                nm = CQ * QG
                csl = slice(c * 2 * nm, (c + 1) * 2 * nm)
                st["w"] = emit_apply(st, c, bt_all[:, csl],
                                     btb_all[:, csl], None)
                if DBG and c == 0:
                    nc.sync.dma_start(dbg["d_w0"][:], st["w"][:, 0:64])

            def ph_fr(st, c, j):
                nm = CQ * QG
                csl = slice(c * 2 * nm, (c + 1) * 2 * nm)
                w = st["w"]
                u1 = fint.tile([128, 2 * nm], f32, tag="u1")
                mul(E_U1, u1[:], w[:], dv2_all[:, csl])
                t2 = fint.tile([128, 2 * nm], f32, tag="t2")
                stt(E_T2, t2[:], w[:], -C0, bt_all[:, csl])
                pp = p_thin.tile([128, 2 * nm], f32, tag="t")
                for mi in range(nm):
                    ao = st["ao"][mi // QG]
                    sl = slice((mi % QG) * 128, (mi % QG + 1) * 128)
                    nc.tensor.matmul(pp[:, 2 * mi:2 * mi + 2], ao[:, sl],
                                     u1[:, 2 * mi:2 * mi + 2],
                                     start=(mi == 0), stop=(mi == nm - 1))
                r1 = fint.tile([128, 2 * nm], f32, tag="r1")
                nc.vector.tensor_mul(r1[:], pp[:], dvS_all[:, csl])
                rt = fint.tile([128, 2 * nm], bf, tag="rt")
                (nc.vector if E_RT == "v" else nc.gpsimd).tensor_sub(
                    rt[:], t2[:], r1[:])
                st["w"] = emit_apply(st, c, None, rt, w)

            def ph_fs(st, c):
                nm = CQ * QG
                csl = slice(c * 2 * nm, (c + 1) * 2 * nm)
                ws = lamp.tile([128, 2 * nm], f32, tag="ws")
                nc.vector.tensor_mul(ws[:], st["w"][:], dv2_all[:, csl])
                sums = p_thin.tile([1, 2 * nm], f32, tag="t")
                nc.tensor.matmul(sums[:], ones_col[:], ws[:])
                num = lamp.tile([1, nm], f32, tag="num")
                nc.vector.tensor_add(
                    num[:], sums[0:1, 0:2 * nm:2],
                    q_all[:, c * nm:(c + 1) * nm])
                den = lamp.tile([1, nm], f32, tag="den")
                nc.vector.tensor_scalar_add(den[:], sums[0:1, 1:2 * nm:2],
                                            -1.0)
                rden = lamp.tile([1, nm], f32, tag="rden")
                nc.vector.reciprocal(rden[:], den[:])
                lamneg = lamp.tile([1, nm], f32, tag="lamneg")
                nc.vector.tensor_mul(lamneg[:], num[:], rden[:])
                lamb = lamp.tile([128, nm], f32, tag="lamb")
                nc.gpsimd.partition_broadcast(lamb[:], lamneg[:])
                t1 = lamp.tile([128, nm], f32, tag="t1")
                nc.vector.tensor_mul(t1[:], ws[:, 1:2 * nm:2], lamb[:])
                qc = lamp.tile([128, nm], f32, tag="qc")
                nc.vector.tensor_sub(qc[:], t1[:], ws[:, 0:2 * nm:2])
                nc.sync.dma_start(out[:, c * nm:(c + 1) * nm], qc[:])
                # release cohort tiles
                st["ao"].clear()
                st["es"].clear()
                st["sx"].clear()

            # phase table
            def emit_phase(st, c, ph):
                if ph == 0:
                    ph_pre(st, c)
                elif ph == 1:
                    ph_sqrt(st, c)
                elif ph == 2:
                    ph_erf(st, c)
                elif ph == 3:
                    ph_warm(st, c)
                elif ph < 3 + K_NS:
                    ph_ns(st, c, ph - 4)
                elif ph == 3 + K_NS:
                    ph_fa(st, c)
                elif ph < 4 + K_NS + N_REF:
                    ph_fr(st, c, ph - 4 - K_NS)
                else:
                    ph_fs(st, c)

            NPH = 5 + K_NS + N_REF
            states = [dict() for _ in range(NCOH)]
            total = OFF * (NCOH - 1) + NPH
            for t in range(total):
                for c in range(NCOH):
                    ph = t - OFF * c
                    if 0 <= ph < NPH:
                        emit_phase(states[c], c, ph)

    nc.compile()
    return nc


def _host_pack(eneg, positions, node_attrs, hardness, total_charge,
               atomic_numbers):
    """Precompute per-atom quantities and pack per-core DRAM tensors."""
    f32 = np.float32
    pos = np.ascontiguousarray(positions, dtype=f32).reshape(B_MOL, N_ATOM, 3)
    Z = np.asarray(atomic_numbers).astype(np.int64).reshape(B_MOL, N_ATOM)
    na = np.asarray(node_attrs, dtype=f32).reshape(B_MOL, N_ATOM, -1)
    hard = np.asarray(hardness, dtype=f32)
    e = np.asarray(eneg, dtype=f32).reshape(B_MOL, N_ATOM)
    Q = np.asarray(total_charge, dtype=f32).reshape(B_MOL)

    cov = (0.3 + 0.02 * np.arange(100)).astype(f32)
    r = cov[Z]                                   # [B, n]
    ss2 = (f32(2.0) * r * r).astype(f32)         # 2*sigma
    n2 = (pos * pos).sum(axis=2, dtype=f32).astype(f32)
    aidx = na.argmax(axis=2)
    dv = (hard[aidx] + f32(1.0) / (np.sqrt(np.pi).astype(f32) * r)).astype(f32)
    shat = (f32(1.0) / np.sqrt(dv)).astype(f32)  # [B, n]

    import ml_dtypes

    def to_bf16(x):
        x = np.ascontiguousarray(x, dtype=np.float32)
        v = x.view(np.uint32).copy()
        v += 0x8000
        v >>= 16
        return np.ascontiguousarray(
            v.astype(np.uint16).view(ml_dtypes.bfloat16))

    mpc = MPC
    in_maps = []
    for c in range(N_CORES):
        sl = slice(c * mpc, (c + 1) * mpc)
        p = pos[sl]          # [mpc, 128, 3]
        nn2 = n2[sl]
        ssl = ss2[sl]
        sh = shat[sl]        # [mpc, 128]
        lhsp = np.zeros((7, mpc, N_ATOM), dtype=f32)
        lhsp[0] = -2.0 * p[:, :, 0]
        lhsp[1] = -2.0 * p[:, :, 1]
        lhsp[2] = -2.0 * p[:, :, 2]
        lhsp[3] = nn2 + EPS_D2
        lhsp[4] = 1.0
        lhsp[5] = ssl
        lhsp[6] = 1.0
        rhsp = np.zeros((7, mpc, 2 * N_ATOM), dtype=f32)
        rhsp[0, :, :N_ATOM] = p[:, :, 0]
        rhsp[1, :, :N_ATOM] = p[:, :, 1]
        rhsp[2, :, :N_ATOM] = p[:, :, 2]
        rhsp[3, :, :N_ATOM] = 1.0
        rhsp[4, :, :N_ATOM] = nn2
        rhsp[5, :, N_ATOM:] = 1.0      # * lhs5 = 2 sig_i
        rhsp[6, :, N_ATOM:] = ssl      # * lhs6 = 1 -> 2 sig_j
        # sst: [128, mpc*128] bf16, col m*128+j, row i = s_i s_j / S0
        sstp = np.einsum("mi,mj->imj", sh, sh).astype(f32) / f32(S0)
        btpk = np.empty((N_ATOM, 2 * mpc), dtype=f32)
        btpk[:, 0::2] = (e[sl] * sh / f32(S0)).T
        btpk[:, 1::2] = (sh / f32(S0)).T
        dv2p = np.empty((N_ATOM, 2 * mpc), dtype=f32)
        dv2p[:, 0::2] = sh.T
        dv2p[:, 1::2] = sh.T
        dvSp = (dv2p / f32(S0)).astype(f32)
        qp = np.ascontiguousarray(Q[sl]).reshape(1, mpc)
        in_maps.append({
            "lhs_pack": np.ascontiguousarray(lhsp.reshape(7, mpc * N_ATOM)),
            "rhs_pack": np.ascontiguousarray(
                rhsp.reshape(7, mpc * 2 * N_ATOM)),
            "sst_pack": to_bf16(sstp.reshape(N_ATOM, mpc * N_ATOM)),
            "bt_pack": btpk,
            "dvs2_pack": dv2p,
            "dvsS2_pack": dvSp,
            "q_pack": qp,
        })
    return in_maps


def run_device(in_maps, trace=False, **kw):
    if "nc" not in _CACHE:
        _CACHE["nc"] = _build_bass()
    nc = _CACHE["nc"]
    return run_bass_kernel_spmd(nc, in_maps, list(range(N_CORES)),
                                trace=trace, **kw)


def kernel(eneg, positions, node_attrs, hardness, total_charge, batch,
           atomic_numbers):
    in_maps = _host_pack(eneg, positions, node_attrs, hardness, total_charge,
                         atomic_numbers)
    res = run_device(in_maps)
    outs = []
    for c in range(N_CORES):
        o = res.results[c]["out"]                # [atom, mol]
        outs.append(np.ascontiguousarray(o.T))   # [mol, atom]
    full = np.concatenate(outs, axis=0).reshape(-1).astype(np.float32)
    return full
